# revision 34
# baseline (speedup 1.0000x reference)
"""Distributed Trainium2 attention kernel for nn_Attention_62337155334878.

Sharding: 8 cores = 2 batches x 4 head-groups (4 heads each).
Per core pipeline (b fixed, 4 heads = 2 pairs):
  MM1: qT/kT = w_qkv_slice @ x.T (PE, bf16); V = x @ w_v.T directly in
       [token, channel] layout (no transposes needed).
  Attention per (pair, 512-query window, key chunk k):
       MM2: scores for both heads issued as concurrent 64-row PE tiles
            (head0 on rows 0-63, head1 on rows 64-127), pair-packed into
            one 2-bank PSUM slot.
       exp: single ACT instruction over the [128, 1024] slot.
       MM3: attn @ V_aug (64 V cols + 64 ones cols -> rowsum) per head,
            accumulated over k; normalize on DVE at window end.
  MM4: out = attnT.T @ w_out streamed per token tile during pair-1 phase.
MM1 remainder / V / MM4 are paced into the ACT-bound attention loop as
fill thunks.  Host sums the 4 partials per batch and adds b_out.
"""

import sys

sys.path.insert(0, "/opt/trn_rl_repo")

import numpy as np

N_CORES = 8
HEADS = 16
B, N, DIM = 2, 2048, 1024
D = DIM // HEADS  # 64
LOCAL_HEADS = HEADS // 4  # 4 heads per core
C_LOC = LOCAL_HEADS * D  # 256
SCALE = HEADS ** -0.5  # 0.25 (reference scales by heads**-0.5)

NT = 16  # key chunks of 128
FT = 8  # dim contraction chunks of 128
NW = 4  # query windows per pair
W = 512  # window width (queries)
LAG = 6  # mm3 lags mm2/exp by this many k-iterations
FILL_NS = 400  # fill budget per iteration (ns of PE time)

LAST_EXEC_TIME_NS = None
_COMPILED = None


def _build():
    import heapq

    import concourse.bass as bass
    import concourse.tile as tile
    from concourse import bacc, mybir

    f32 = mybir.dt.float32
    bf16 = mybir.dt.bfloat16
    Exp = mybir.ActivationFunctionType.Exp
    ts = bass.ts

    nc = bacc.Bacc("TRN2", target_bir_lowering=False, debug=False,
                   num_devices=N_CORES)

    # xt arrives as [4 quarters][128 partitions][8 fts x 512] blocks: one
    # dense 1MB DMA per quarter with 8KB per-partition lines
    xt_d = nc.dram_tensor("xt", [4 * 128, FT * 512], bf16,
                          kind="ExternalInput").ap()
    # wqkvt pre-blocked to [128 partitions][8 fts x 768]: one 1.5MB DMA
    wqkvt_d = nc.dram_tensor("wqkvt", [128, FT * 3 * C_LOC], bf16,
                             kind="ExternalInput").ap()
    woutt_d = nc.dram_tensor("woutt", [C_LOC, DIM], bf16,
                             kind="ExternalInput").ap()
    out_d = nc.dram_tensor("out", [N, DIM], bf16, kind="ExternalOutput").ap()

    # wqkvt columns: Q01 0:128, Q23 128:256, K01 256:384, K23 384:512,
    # V(all 4 heads) 512:768.  qkvt ct: 0=Q01, 1=K01, 2=Q23, 3=K23.
    WQ_COLS = {0: 0, 1: 256, 2: 128, 3: 384}

    with tile.TileContext(nc) as tc:
        with (
            tc.tile_pool(name="w", bufs=1) as wpool,
            tc.tile_pool(name="x", bufs=1) as xpool,
            tc.tile_pool(name="qk", bufs=1) as qkpool,
            tc.tile_pool(name="va", bufs=1) as vapool,
            tc.tile_pool(name="pt", bufs=1) as ptpool,
            tc.tile_pool(name="at", bufs=1) as atpool,
            tc.tile_pool(name="nrm", bufs=4) as npool,
            tc.tile_pool(name="ob", bufs=2) as opool,
            tc.tile_pool(name="ps2", bufs=2, space="PSUM") as ps2p,
            tc.tile_pool(name="ps3", bufs=2, space="PSUM") as ps3p,
            tc.tile_pool(name="psf", bufs=2, space="PSUM") as psfp,
        ):
            # ---- SBUF tiles ----
            # xt_sb is quarter-major: [128, quarter, ft, 512]
            xt_sb = xpool.tile([128, 4, FT, 512], bf16)
            wq_sb = wpool.tile([128, FT, 3 * C_LOC], bf16, tag="wq")
            wo_sb = wpool.tile([128, 2, DIM], bf16, tag="wo")
            qkvt = qkpool.tile([128, 4, N], bf16)
            v_aug = vapool.tile([128, NT, LOCAL_HEADS, 128], bf16)
            pt = ptpool.tile([128, NT, 1024], bf16)
            attnt = atpool.tile([128, 2, N], bf16)

            nc.vector.memset(v_aug[:], 1.0)

            # ---- loads (hwdge queues: sync + scalar, scalar only at t=0)
            nc.scalar.dma_start(
                out=wq_sb[:].rearrange("p a b -> p (a b)"), in_=wqkvt_d[:])
            # token-major: quarter q (all 8 ft chunks of tokens 512q..)
            # arrives together so attention window 0 paces with the load
            for q in range(4):
                nc.sync.dma_start(
                    out=xt_sb[:, q, :, :].rearrange("p a b -> p (a b)"),
                    in_=xt_d[ts(q, 128), :])
            for i in range(2):
                nc.scalar.dma_start(out=wo_sb[:, i, :],
                                    in_=woutt_d[ts(i, 128), :])

            # ---------- fill thunk generators (each thunk <= ~430ns PE) ----
            def qk_chunk_thunks(ct, c):
                st = {}

                def start():
                    st["ps"] = psfp.tile([128, 512], f32, tag="psf",
                                         name=f"qk{ct}_{c}")

                def step(ft):
                    nc.tensor.matmul(
                        st["ps"][:],
                        wq_sb[:, ft, WQ_COLS[ct]:WQ_COLS[ct] + 128],
                        xt_sb[:, c, ft, :],
                        start=(ft == 0), stop=(ft == FT - 1))

                def drain():
                    nc.vector.tensor_copy(out=qkvt[:, ct, ts(c, 512)],
                                          in_=st["ps"][:])

                th = [(start, 0)]
                th += [(lambda f=f: step(f), 216) for f in range(FT)]
                th += [(drain, 0)]
                return th

            def v_chunk_thunks(tt):
                st = {}

                def start():
                    st["ps"] = psfp.tile([128, 512], f32, tag="psf",
                                         name=f"v{tt}")

                def step(ft):
                    t4 = 128 * (tt % 4)
                    nc.tensor.matmul(
                        st["ps"][:, 0:256],
                        xt_sb[:, tt // 4, ft, t4:t4 + 128],
                        wq_sb[:, ft, 512:768],
                        start=(ft == 0), stop=(ft == FT - 1))

                def drain():
                    nc.vector.tensor_copy(
                        out=v_aug[:, tt, :, 0:64],
                        in_=st["ps"][:, 0:256].rearrange(
                            "p (h d) -> p h d", d=64))
                    v_done[0] += 1

                th = [(start, 0)]
                th += [(lambda f=f: step(f), 120) for f in range(FT)]
                th += [(drain, 0)]
                return th

            def mm4_thunks(tt):
                st = {}

                def start():
                    st["o"] = opool.tile([128, 1024], bf16, tag="osb",
                                         name=f"osb{tt}")

                def half_start(oc):
                    st[oc] = psfp.tile([128, 512], f32, tag="psf",
                                       name=f"o{tt}_{oc}")
                    nc.tensor.matmul(st[oc][:], attnt[:, 0, ts(tt, 128)],
                                     wo_sb[:, 0, ts(oc, 512)],
                                     start=True, stop=False)

                def half_end(oc):
                    nc.tensor.matmul(st[oc][:], attnt[:, 1, ts(tt, 128)],
                                     wo_sb[:, 1, ts(oc, 512)],
                                     start=False, stop=True)

                def half_drain(oc):
                    nc.vector.tensor_copy(out=st["o"][:, ts(oc, 512)],
                                          in_=st[oc][:])
                    if oc == 1:
                        eng = nc.sync if tt % 2 == 0 else nc.scalar
                        eng.dma_start(out=out_d[ts(tt, 128), :],
                                      in_=st["o"][:])

                th = [(start, 0)]
                for oc in range(2):
                    th += [(lambda o=oc: half_start(o), 216),
                           (lambda o=oc: half_end(o), 216),
                           (lambda o=oc: half_drain(o), 0)]
                return th

            # ---------- attention machinery ----------
            TOT = 2 * NW * NT  # 128 iterations

            def pwk(i):
                # pairs interleaved per window: p0w0, p1w0, p0w1, ...
                # so out-projection work spreads across the whole run
                idx = i // NT
                return idx % 2, idx // 2, i % NT

            slots = {}
            ps3s = {}
            fills_heap = []
            cur_chunk = []
            cur_deadline = [0]
            seq = [0]
            spent = [0.0]
            cur_iter = [0]
            v_done = [0]

            def emit_mm2(i):
                p, w, k = pwk(i)
                slot = ps2p.tile([128, 1024], f32, tag="ps2", name=f"s{i}")
                slots[i] = slot
                qct, kct = 2 * p, 2 * p + 1
                for j in range(2):
                    nc.tensor.matmul(
                        slot[:, ts(j, 512)],
                        qkvt[64 * j:64 * j + 64, kct, ts(k, 128)],
                        qkvt[64 * j:64 * j + 64, qct, ts(w, 512)],
                        start=True, stop=True)

            def emit_exp(i):
                p, w, k = pwk(i)
                nc.scalar.activation(pt[:, k, :], slots.pop(i)[:], Exp,
                                     scale=SCALE)

            def emit_norm(p, w):
                if (p, w) == (1, NW - 1):
                    # finale: per-token-tile norm + immediate out-proj so
                    # the tail is one tile deep, not a whole window
                    for tq in range(4):
                        for j in range(2):
                            ps3 = ps3s[(p, w)][j]
                            rb = npool.tile([64, 128], f32, tag="rb4")
                            nc.vector.tensor_copy(
                                out=rb[:], in_=ps3[64:128, ts(tq, 128)])
                            rinv = npool.tile([64, 128], f32, tag="rinv4")
                            nc.vector.reciprocal_approx_fast(rinv[:], rb[:])
                            c0 = 512 * w + 128 * tq
                            nc.vector.tensor_mul(
                                attnt[64 * j:64 * j + 64, p, c0:c0 + 128],
                                ps3[0:64, ts(tq, 128)], rinv[:])
                        for fn, _ in mm4_thunks(4 * w + tq):
                            fn()
                    return
                for j in range(2):
                    ps3 = ps3s[(p, w)][j]
                    rb = npool.tile([64, 512], f32, tag="rb")
                    nc.vector.tensor_copy(out=rb[:], in_=ps3[64:128, :])
                    rinv = npool.tile([64, 512], f32, tag="rinv")
                    nc.vector.reciprocal_approx_fast(rinv[:], rb[:])
                    nc.vector.tensor_mul(
                        attnt[64 * j:64 * j + 64, p, ts(w, 512)],
                        ps3[0:64, :], rinv[:])
                if p == 1:
                    for t4 in range(4):
                        add_fills(mm4_thunks(4 * w + t4),
                                  cur_iter[0] + 2 + 4 * t4)

            def emit_mm3(i):
                p, w, k = pwk(i)
                if k == 0:
                    ps3s[(p, w)] = [
                        ps3p.tile([128, 512], f32, tag="ps3",
                                  name=f"ps3_{p}_{w}_{j}")
                        for j in range(2)
                    ]
                for j in range(2):
                    nc.tensor.matmul(
                        ps3s[(p, w)][j][:],
                        v_aug[:, k, 2 * p + j, :],
                        pt[:, k, ts(j, 512)],
                        start=(k == 0), stop=(k == NT - 1))
                if k == NT - 1:
                    emit_norm(p, w)

            def add_fills(thunks, deadline):
                # whole chunks keyed by deadline; a chunk is never
                # interleaved with another (bounds psf pool liveness)
                seq[0] += 1
                heapq.heappush(fills_heap, (deadline, seq[0], list(thunks)))

            def pop_fills(i):
                budget = FILL_NS * (i + 1)
                while True:
                    if not cur_chunk:
                        if not fills_heap:
                            return
                        d, _, th = fills_heap[0]
                        if d > i + 4 and spent[0] >= budget:
                            return
                        heapq.heappop(fills_heap)
                        cur_chunk.extend(th)
                        cur_deadline[0] = d
                    while cur_chunk:
                        if cur_deadline[0] > i + 4 and spent[0] >= budget:
                            return
                        fn, cost = cur_chunk.pop(0)
                        fn()
                        spent[0] += cost

            # ---- pre-phase: only the chunk-0 work MM2 iter 0 needs ----
            for ct in (0, 1):
                for fn, _ in qk_chunk_thunks(ct, 0):
                    fn()

            # ---- fill inventory (deadline = iter by which it must run;
            # window 0 is load-paced so its fills get natural slack) ----
            staged = []
            for c in range(1, 4):  # K01: (p0,w0) keys 4c @ iter 4c
                staged.append((qk_chunk_thunks(1, c), 4 * c + 2))
            for tt in range(NT):  # v_aug[k] for mm3 job k @ iter k+LAG
                staged.append((v_chunk_thunks(tt), tt + 3))
            for c in range(4):  # K23: (p1,w0) keys 4c @ iter 16+4c
                staged.append((qk_chunk_thunks(3, c), 12 + 4 * c))
            staged.append((qk_chunk_thunks(2, 0), 12))  # Q23 w0 @ 16
            for c in range(1, 4):  # Q01 window c @ iter 32c
                staged.append((qk_chunk_thunks(0, c), 32 * c - 4))
            for c in range(1, 4):  # Q23 window c @ iter 32c+16
                staged.append((qk_chunk_thunks(2, c), 32 * c + 12))
            staged.sort(key=lambda e: e[1])
            for th, d in staged:
                add_fills(th, d)

            # ---- main attention loop (2-iteration groups) ----
            for i in range(0, TOT, 2):
                emit_mm2(i)
                emit_mm2(i + 1)
                emit_exp(i)
                emit_exp(i + 1)
                if i - LAG >= 0:
                    emit_mm3(i - LAG)
                if i + 1 - LAG >= 0:
                    emit_mm3(i + 1 - LAG)
                cur_iter[0] = i + 1
                pop_fills(i + 1)

            # ---- drain: flush pending fills first (the finale emits
            # mm4 inline and must not interleave with a partial chunk) ----
            while cur_chunk or fills_heap:
                if not cur_chunk:
                    cur_chunk.extend(heapq.heappop(fills_heap)[2])
                fn, _ = cur_chunk.pop(0)
                fn()
            for i in range(TOT - LAG, TOT):
                emit_mm3(i)

    nc.compile()
    return nc


def _get_compiled():
    global _COMPILED
    if _COMPILED is None:
        _COMPILED = _build()
    return _COMPILED


def _np_reference(x, mask, w_qkv, w_out, b_out):
    b, n, dim = x.shape
    h = HEADS
    d = dim // h
    qkv = x @ w_qkv.T
    qkv = qkv.reshape(b, n, 3, h, d)
    q = np.moveaxis(qkv[:, :, 0], 2, 1)
    k = np.moveaxis(qkv[:, :, 1], 2, 1)
    v = np.moveaxis(qkv[:, :, 2], 2, 1)
    dots = np.einsum("bhid,bhjd->bhij", q, k) * (h ** -0.5)
    m = np.concatenate([np.ones((b, 1), dtype=bool), mask], axis=1)
    pair = m[:, None, :] & m[:, :, None]
    dots = np.where(pair[:, None, :, :], dots, -np.inf)
    dots = dots - dots.max(axis=-1, keepdims=True)
    e = np.exp(dots)
    attn = e / e.sum(axis=-1, keepdims=True)
    out = np.einsum("bhij,bhjd->bhid", attn, v)
    out = np.moveaxis(out, 1, 2).reshape(b, n, dim)
    return (out @ w_out.T + b_out).astype(np.float32)


def kernel(x, mask, w_qkv, w_out, b_out, _trace=False):
    x = np.asarray(x, dtype=np.float32)
    mask = np.asarray(mask)
    w_qkv = np.asarray(w_qkv, dtype=np.float32)
    w_out = np.asarray(w_out, dtype=np.float32)
    b_out = np.asarray(b_out, dtype=np.float32)

    if not bool(mask.all()):
        # spec fills mask with ones; fall back to host math if it ever isn't
        return _np_reference(x, mask, w_qkv, w_out, b_out)

    from concourse.bass_utils import run_bass_kernel_spmd

    nc = _get_compiled()

    in_maps = []
    for core in range(N_CORES):
        b = core // 4
        g = core % 4
        h0 = g * LOCAL_HEADS
        rows = []
        for part in range(3):  # q, k, v row-blocks of w_qkv
            r0 = part * DIM + h0 * D
            rows.append(w_qkv[r0:r0 + C_LOC])
        w_slice = np.concatenate(rows, axis=0)  # [768, 1024]
        import ml_dtypes
        xtq = np.ascontiguousarray(
            x[b].T.reshape(8, 128, 4, 512).transpose(2, 1, 0, 3)
        ).reshape(4 * 128, 8 * 512)
        wqb = np.ascontiguousarray(
            w_slice.T.reshape(8, 128, 768).transpose(1, 0, 2)
        ).reshape(128, 8 * 768)
        in_maps.append({
            "xt": xtq.astype(ml_dtypes.bfloat16),
            "wqkvt": wqb.astype(ml_dtypes.bfloat16),
            "woutt": np.ascontiguousarray(
                w_out[:, h0 * D:h0 * D + C_LOC].T).astype(ml_dtypes.bfloat16),
        })

    global LAST_EXEC_TIME_NS
    if _trace:
        _install_profhook()
        res = run_bass_kernel_spmd(nc, in_maps, list(range(N_CORES)),
                                   trace=True)
        LAST_EXEC_TIME_NS = res.exec_time_ns
    else:
        res = run_bass_kernel_spmd(nc, in_maps, list(range(N_CORES)),
                                   trace=False)

    out = np.zeros((B, N, DIM), dtype=np.float32)
    for core in range(N_CORES):
        out[core // 4] += res.results[core]["out"].astype(np.float32)
    out += b_out
    return out


def _install_profhook():
    import contextlib
    import ctypes
    import types

    if "antenv.axon_hooks" in sys.modules:
        return
    lib = ctypes.CDLL("/opt/axon/libaxon_pjrt.so")
    if not hasattr(lib, "axon_start_nrt_profile"):
        return
    lib.axon_start_nrt_profile.argtypes = [ctypes.POINTER(ctypes.c_int64),
                                           ctypes.c_size_t]
    lib.axon_start_nrt_profile.restype = ctypes.c_int64
    lib.axon_stop_nrt_profile.argtypes = [ctypes.c_char_p]
    lib.axon_stop_nrt_profile.restype = ctypes.c_int64

    @contextlib.contextmanager
    def _hook_cm(output_dir, device_ids):
        import jax

        jax.devices()
        if device_ids:
            ids = (ctypes.c_int64 * len(device_ids))(*device_ids)
            rc = lib.axon_start_nrt_profile(ids, len(device_ids))
        else:
            rc = lib.axon_start_nrt_profile(None, 0)
        if rc != 0:
            raise RuntimeError(f"axon_start_nrt_profile rc={rc}")
        try:
            yield
        finally:
            n = lib.axon_stop_nrt_profile(str(output_dir).encode())
            print(f"profile: {n} file(s) written to {output_dir}",
                  file=sys.stderr)

    mod = types.ModuleType("antenv.axon_hooks")
    mod.get_axon_ntff_profile_hook = lambda: _hook_cm
    mod.set_axon_ntff_profile_hook = lambda h: None
    sys.modules["antenv.axon_hooks"] = mod
    import antenv

    antenv.axon_hooks = mod
